# revision 17
# baseline (speedup 1.0000x reference)
"""Trainium2 Bass kernel for nn_CrossAttentionBlock_73452530696666.

Math note: the reference's attention softmax runs over a single KV token, so
attn == 1.0 exactly and the whole q/scores path is dead code. The output
reduces to, per batch b and spatial position s:

    p[b]   = (text_emb[b] @ Wv.T) @ Wo.T + bo          # (C,) per batch
    y[:,s] = LayerNorm_C(x[:, s] + p[b]) * gamma + beta

So the kernel is a tiny pair of per-batch matvecs plus a fused bias-add +
LayerNorm over the channel dim streamed over the full (B, C, H*W) tensor.

Sharding: data-parallel over batch, 2 batches per core on 8 cores. Layout
keeps C on partitions (4 chunks of 128) so all DMA is contiguous; channel
reductions (mean / mean-of-squares) run on the TensorEngine via ones-vector
matmuls; (x - mu) is formed in PSUM by an identity matmul plus a K=1
broadcast matmul; the final (hm + p) * rstd is one fused scalar_tensor_tensor
per tile on the VectorEngine.
"""

import sys

sys.path.insert(0, "/opt/trn_rl_repo")

import numpy as np

B, C, H, W, T = 16, 512, 64, 64, 768
S = H * W  # 4096
NCORES = 8
BPC = B // NCORES  # batches per core = 2
NCH = C // 128  # channel chunks = 4
MACRO = 2048  # spatial columns per macro tile (DMA/ACT granularity)
SUB = 512  # matmul / PSUM sub tile
NSUB = MACRO // SUB  # 4
NMACRO = S // MACRO  # 2 per batch
EPS = 1e-5

# Set by test harness to request a profiled run.
TRACE = False
LAST_RESULTS = None

_CACHE = {}


def _build(trivial_affine: bool, dual_psum_stt: bool = True):
    import concourse.bass as bass
    import concourse.tile as tile
    from concourse import bacc, mybir
    from concourse.masks import make_identity

    f32 = mybir.dt.float32
    bf16 = mybir.dt.float16
    AF = mybir.ActivationFunctionType
    OP = mybir.AluOpType
    NTC = T // 128  # text-emb chunks = 6

    nc = bacc.Bacc("TRN2", target_bir_lowering=False)
    x = nc.dram_tensor("x", (BPC, C, S), f32, kind="ExternalInput")
    teT = nc.dram_tensor("teT", (T, BPC), f32, kind="ExternalInput")
    wvT = nc.dram_tensor("wvT", (T, C), f32, kind="ExternalInput")
    woT = nc.dram_tensor("woT", (C, C), f32, kind="ExternalInput")
    bo = nc.dram_tensor("bo", (1, C), f32, kind="ExternalInput")
    if not trivial_affine:
        gcols = nc.dram_tensor("gcols", (128, NCH), f32, kind="ExternalInput")
        bcols = nc.dram_tensor("bcols", (128, NCH), f32, kind="ExternalInput")
    y = nc.dram_tensor("y", (BPC, C, S), f32, kind="ExternalOutput")

    xv = x.rearrange("b (n p) s -> b p n s", p=128)
    yv = y.rearrange("b (n p) s -> b p n s", p=128)

    with tile.TileContext(nc) as tc:
        with tc.tile_pool(name="consts", bufs=1) as consts:
            # ---------------- constants ----------------
            ident = consts.tile([128, 128], f32)
            make_identity(nc, ident)
            ident16 = consts.tile([128, 128], bf16)
            make_identity(nc, ident16)
            ones_c = consts.tile([128, 1], bf16)
            nc.vector.memset(ones_c, 1.0 / C)  # lhsT for channel-mean matmuls
            ones_r = consts.tile([1, 128], f32)
            nc.vector.memset(ones_r, 1.0)  # lhsT for K=1 broadcast matmuls
            ones_r16 = consts.tile([1, 128], bf16)
            nc.vector.memset(ones_r16, 1.0)
            eps1 = consts.tile([1, 1], f32)
            nc.vector.memset(eps1, EPS)
            pcol_sb = consts.tile([128, NCH, BPC], f32)
            nbias_row = consts.tile([1, BPC], f32)

            # ---------------- phase 0: p = (te @ Wv.T) @ Wo.T + bo ----------------
            with (
                tc.tile_pool(name="wpool", bufs=1) as wpool,
                tc.tile_pool(name="p0p", bufs=1, space="PSUM") as p0p,
            ):
                te_sb = wpool.tile([128, NTC, BPC], f32)
                nc.sync.dma_start(te_sb, teT.rearrange("(n p) b -> p n b", p=128))
                wv_sb = wpool.tile([128, NTC, C], f32)
                nc.sync.dma_start(wv_sb, wvT.rearrange("(n p) c -> p n c", p=128))
                wo_sb = wpool.tile([128, NCH, C], f32)
                nc.sync.dma_start(wo_sb, woT.rearrange("(n p) c -> p n c", p=128))
                bo_sb = wpool.tile([BPC, C], f32)
                nc.sync.dma_start(bo_sb, bo[:, :].to_broadcast((BPC, C)))

                p1_ps = p0p.tile([BPC, C], f32, tag="p0")
                for n in range(NTC):
                    nc.tensor.matmul(
                        p1_ps, te_sb[:, n, :], wv_sb[:, n, :],
                        start=(n == 0), stop=(n == NTC - 1),
                    )
                p1_sb = wpool.tile([BPC, C], f32)
                nc.scalar.copy(p1_sb, p1_ps)

                p1t_sb = wpool.tile([128, NCH, BPC], f32)
                for ci in range(NCH):
                    t_ps = p0p.tile([128, BPC], f32, tag="p0")
                    nc.tensor.transpose(
                        t_ps, p1_sb[:, ci * 128:(ci + 1) * 128], ident[:BPC, :BPC]
                    )
                    nc.scalar.copy(p1t_sb[:, ci, :], t_ps)

                p2_ps = p0p.tile([BPC, C], f32, tag="p0")
                for ci in range(NCH):
                    nc.tensor.matmul(
                        p2_ps, p1t_sb[:, ci, :], wo_sb[:, ci, :],
                        start=(ci == 0), stop=(ci == NCH - 1),
                    )
                p_sb = wpool.tile([BPC, C], f32)
                nc.vector.tensor_add(p_sb, p2_ps, bo_sb)

                # p as columns: pcol_sb[:, ci, b] = p[b, ci*128 + :]
                for ci in range(NCH):
                    t_ps = p0p.tile([128, BPC], f32, tag="p0")
                    nc.tensor.transpose(
                        t_ps, p_sb[:, ci * 128:(ci + 1) * 128], ident[:BPC, :BPC]
                    )
                    nc.scalar.copy(pcol_sb[:, ci, :], t_ps)

                # -c1b row: -(sum_c p[b, c]) / C, shape (1, BPC)
                csum = wpool.tile([BPC, 1], f32)
                nc.vector.reduce_sum(csum, p_sb, axis=mybir.AxisListType.X)
                c1t_ps = p0p.tile([1, BPC], f32, tag="p0")
                nc.tensor.transpose(c1t_ps, csum, ident[:BPC, :BPC])
                nc.scalar.activation(nbias_row, c1t_ps, AF.Copy, scale=-1.0 / C)

            if not trivial_affine:
                g_sb = consts.tile([128, NCH], f32)
                nc.sync.dma_start(g_sb, gcols)
                b_sb = consts.tile([128, NCH], f32)
                nc.sync.dma_start(b_sb, bcols)

            # ---------------- main loop ----------------
            with (
                tc.tile_pool(name="xp", bufs=2) as xp,
                tc.tile_pool(name="x16p", bufs=2) as x16p,
                tc.tile_pool(name="yp", bufs=2) as yp,
                tc.tile_pool(name="sqp", bufs=5) as sqp,
                tc.tile_pool(name="rowp", bufs=2) as rowp,
                tc.tile_pool(name="mup", bufs=2, space="PSUM") as mup,
                tc.tile_pool(name="e2p", bufs=2, space="PSUM") as e2p,
                tc.tile_pool(name="rstp", bufs=2, space="PSUM") as rstp,
                tc.tile_pool(name="hmp", bufs=2, space="PSUM") as hmp,
            ):
             for b in range(BPC):
                for m in range(NMACRO):
                    s0 = m * MACRO
                    xt = xp.tile([128, NCH, MACRO], f32)
                    nc.sync.dma_start(xt, xv[b, :, :, s0:s0 + MACRO])
                    x16 = x16p.tile([128, NCH, MACRO], bf16)
                    nc.gpsimd.tensor_copy(x16, xt)

                    # squares (x + p)^2 with the p-add folded into the ACT bias
                    sq_tiles = []
                    for ci in range(NCH):
                        sq = sqp.tile([128, MACRO], bf16, name=f"sq{ci}", tag="sq")
                        nc.scalar.activation(
                            sq, xt[:, ci, :], AF.Square,
                            bias=pcol_sb[:, ci, b:b + 1], scale=1.0,
                        )
                        sq_tiles.append(sq)

                    yt = yp.tile([128, NCH, MACRO], f32)

                    for j in range(NSUB):
                        sl = slice(SUB * j, SUB * (j + 1))
                        # channel sums -> mu', channel sums of squares -> E2
                        mu_ps = mup.tile([1, SUB], f32)
                        e2_ps = e2p.tile([1, SUB], f32)
                        for ci in range(NCH):
                            nc.tensor.matmul(
                                mu_ps, ones_c, x16[:, ci, sl],
                                start=(ci == 0), stop=(ci == NCH - 1),
                            )
                        for ci in range(NCH):
                            nc.tensor.matmul(
                                e2_ps, ones_c, sq_tiles[ci][:, sl],
                                start=(ci == 0), stop=(ci == NCH - 1),
                            )
                        # stats finalize on rows (w is reused in place:
                        # mu^2 -> var -> sd -> rstd)
                        negmu = rowp.tile([1, SUB], bf16, tag="negmu")
                        nc.scalar.activation(
                            negmu, mu_ps, AF.Identity,
                            scale=-1.0, bias=nbias_row[:, b:b + 1],
                        )
                        w = rowp.tile([1, SUB], f32, tag="w")
                        nc.scalar.activation(w, negmu, AF.Square)
                        nc.vector.tensor_tensor(w, e2_ps, w, op=OP.subtract)
                        nc.scalar.activation(w, w, AF.Sqrt, bias=eps1)
                        rstd = rowp.tile([1, SUB], f32, tag="rstd")
                        rscr = rowp.tile([1, SUB], f32, tag="rscr")
                        nc.vector.reciprocal_approx_accurate(rstd, w, scratch=rscr)
                        # broadcast rstd across partitions, then move to SBUF
                        # (TensorScalarPtr may read only one PSUM input)
                        rst_ps = rstp.tile([128, SUB], f32)
                        nc.tensor.matmul(rst_ps, ones_r, rstd, start=True, stop=True)
                        rst_sb = rowp.tile([128, SUB], f32, tag="rst_sb")
                        nc.scalar.copy(rst_sb, rst_ps)

                        for ci in range(NCH):
                            # hm = x - mu  (identity matmul + K=1 broadcast accumulate)
                            hm_ps = hmp.tile([128, SUB], f32)
                            nc.tensor.matmul(
                                hm_ps, ident16, x16[:, ci, sl], start=True, stop=False
                            )
                            nc.tensor.matmul(
                                hm_ps, ones_r16, negmu, start=False, stop=True
                            )
                            out_sl = yt[:, ci, sl]
                            # y = (hm + p) * rstd in one fused vector op
                            nc.vector.scalar_tensor_tensor(
                                out_sl, hm_ps, pcol_sb[:, ci, b:b + 1], rst_sb,
                                op0=OP.add, op1=OP.mult,
                            )
                            if not trivial_affine:
                                nc.vector.tensor_scalar(
                                    out_sl, out_sl,
                                    g_sb[:, ci:ci + 1], b_sb[:, ci:ci + 1],
                                    op0=OP.mult, op1=OP.add,
                                )

                    nc.sync.dma_start(yv[b, :, :, s0:s0 + MACRO], yt)

    nc.compile()
    return nc


def _get_module(trivial_affine: bool):
    key = trivial_affine
    if key not in _CACHE:
        _CACHE[key] = _build(trivial_affine)
    return _CACHE[key]


def kernel(**inputs) -> np.ndarray:
    global LAST_RESULTS
    from concourse.bass_utils import run_bass_kernel_spmd

    x = np.ascontiguousarray(np.asarray(inputs["x"], dtype=np.float32))
    te = np.asarray(inputs["text_emb"], dtype=np.float32)
    Wv = np.asarray(inputs["Wv"], dtype=np.float32)
    Wo = np.asarray(inputs["Wo"], dtype=np.float32)
    bo = np.asarray(inputs["bo"], dtype=np.float32)
    gamma = np.asarray(inputs["gamma"], dtype=np.float32)
    beta = np.asarray(inputs["beta"], dtype=np.float32)
    assert x.shape == (B, C, H, W), x.shape

    trivial = bool(np.all(gamma == 1.0) and np.all(beta == 0.0))
    nc = _get_module(trivial)

    xr = x.reshape(B, C, S)
    teT = np.ascontiguousarray(te.T)  # (T, B)
    wvT = np.ascontiguousarray(Wv.T)  # (T, C)
    woT = np.ascontiguousarray(Wo.T)  # (C, C)
    bo2 = np.ascontiguousarray(bo.reshape(1, C))

    in_maps = []
    for c in range(NCORES):
        m = {
            "x": np.ascontiguousarray(xr[BPC * c:BPC * (c + 1)]),
            "teT": np.ascontiguousarray(teT[:, BPC * c:BPC * (c + 1)]),
            "wvT": wvT,
            "woT": woT,
            "bo": bo2,
        }
        if not trivial:
            m["gcols"] = np.ascontiguousarray(gamma.reshape(NCH, 128).T)
            m["bcols"] = np.ascontiguousarray(beta.reshape(NCH, 128).T)
        in_maps.append(m)

    kwargs = {}
    if TRACE:
        import os

        os.makedirs("/tmp/bassprof", exist_ok=True)
        kwargs["tmpdir"] = "/tmp/bassprof"
    res = run_bass_kernel_spmd(
        nc, in_maps, core_ids=list(range(NCORES)), trace=TRACE, **kwargs
    )
    LAST_RESULTS = res
    out = np.concatenate([res.results[c]["y"] for c in range(NCORES)], axis=0)
    return np.ascontiguousarray(out.reshape(B, C, H, W).astype(np.float32))


# revision 20
# speedup vs baseline: 1.4447x; 1.4447x over previous
"""Trainium2 Bass kernel for nn_CrossAttentionBlock_73452530696666.

Math note: the reference's attention softmax runs over a single KV token, so
attn == 1.0 exactly and the whole q/scores path is dead code. The output
reduces to, per batch b and spatial position s:

    p[b]   = (text_emb[b] @ Wv.T) @ Wo.T + bo          # (C,) per batch
    y[:,s] = LayerNorm_C(x[:, s] + p[b]) * gamma + beta

So the kernel is a tiny pair of per-batch matvecs plus a fused bias-add +
LayerNorm over the channel dim streamed over the full (B, C, H*W) tensor.

Sharding: data-parallel over batch, 2 batches per core on 8 cores. Layout
keeps C on partitions (4 chunks of 128) so all DMA is contiguous; channel
reductions (mean / mean-of-squares) run on the TensorEngine via ones-vector
matmuls; (x - mu) is formed in PSUM by an identity matmul plus a K=1
broadcast matmul; the final (hm + p) * rstd is one fused scalar_tensor_tensor
per tile on the VectorEngine.
"""

import sys

sys.path.insert(0, "/opt/trn_rl_repo")

import numpy as np

B, C, H, W, T = 16, 512, 64, 64, 768
S = H * W  # 4096
NCORES = 8
BPC = B // NCORES  # batches per core = 2
NCH = C // 128  # channel chunks = 4
MACRO = 2048  # spatial columns per macro tile (DMA/ACT granularity)
SUB = 512  # matmul / PSUM sub tile
NSUB = MACRO // SUB  # 4
NMACRO = S // MACRO  # 2 per batch
EPS = 1e-5

# Set by test harness to request a profiled run.
TRACE = False
LAST_RESULTS = None

_CACHE = {}


def _build(trivial_affine: bool, dual_psum_stt: bool = True):
    import concourse.bass as bass
    import concourse.tile as tile
    from concourse import bacc, mybir
    from concourse.masks import make_identity

    f32 = mybir.dt.float32
    bf16 = mybir.dt.float16
    AF = mybir.ActivationFunctionType
    OP = mybir.AluOpType
    NTC = T // 128  # text-emb chunks = 6

    nc = bacc.Bacc("TRN2", target_bir_lowering=False)
    x = nc.dram_tensor("x", (BPC, C, S), f32, kind="ExternalInput")
    teT = nc.dram_tensor("teT", (T, BPC), f32, kind="ExternalInput")
    wvT = nc.dram_tensor("wvT", (T, C), f32, kind="ExternalInput")
    woT = nc.dram_tensor("woT", (C, C), f32, kind="ExternalInput")
    bo = nc.dram_tensor("bo", (1, C), f32, kind="ExternalInput")
    if not trivial_affine:
        gcols = nc.dram_tensor("gcols", (128, NCH), f32, kind="ExternalInput")
        bcols = nc.dram_tensor("bcols", (128, NCH), f32, kind="ExternalInput")
    y = nc.dram_tensor("y", (BPC, C, S), f32, kind="ExternalOutput")

    xv = x.rearrange("b (n p) s -> b p n s", p=128)
    yv = y.rearrange("b (n p) s -> b p n s", p=128)

    with tile.TileContext(nc) as tc:
        with tc.tile_pool(name="consts", bufs=1) as consts:
            # ---------------- constants ----------------
            ident = consts.tile([128, 128], f32)
            make_identity(nc, ident)
            ident16 = consts.tile([128, 128], bf16)
            make_identity(nc, ident16)
            ones_c = consts.tile([128, 1], bf16)
            nc.vector.memset(ones_c, 1.0 / C)  # lhsT for channel-mean matmuls
            ones_r = consts.tile([1, 128], f32)
            nc.vector.memset(ones_r, 1.0)  # lhsT for K=1 broadcast matmuls
            ones97_16 = consts.tile([97, 128], bf16)
            nc.vector.memset(ones97_16, 1.0)
            ones97 = consts.tile([97, 128], f32)
            nc.vector.memset(ones97, 1.0)
            ones_m = consts.tile([1, 97], f32)
            nc.vector.memset(ones_m, 1.0)
            eps97 = consts.tile([97, 1], f32)
            nc.vector.memset(eps97, EPS)
            pcol_sb = consts.tile([128, NCH, BPC], f32)
            nbias_row = consts.tile([1, BPC], f32)
            nbias97 = consts.tile([97, BPC], f32)

            # ---------------- phase 0: p = (te @ Wv.T) @ Wo.T + bo ----------------
            with (
                tc.tile_pool(name="wpool", bufs=1) as wpool,
                tc.tile_pool(name="p0p", bufs=1, space="PSUM") as p0p,
            ):
                te_sb = wpool.tile([128, NTC, BPC], f32)
                nc.sync.dma_start(te_sb, teT.rearrange("(n p) b -> p n b", p=128))
                wv_sb = wpool.tile([128, NTC, C], f32)
                nc.sync.dma_start(wv_sb, wvT.rearrange("(n p) c -> p n c", p=128))
                wo_sb = wpool.tile([128, NCH, C], f32)
                nc.sync.dma_start(wo_sb, woT.rearrange("(n p) c -> p n c", p=128))
                bo_sb = wpool.tile([BPC, C], f32)
                nc.sync.dma_start(bo_sb, bo[:, :].to_broadcast((BPC, C)))

                p1_ps = p0p.tile([BPC, C], f32, tag="p0")
                for n in range(NTC):
                    nc.tensor.matmul(
                        p1_ps, te_sb[:, n, :], wv_sb[:, n, :],
                        start=(n == 0), stop=(n == NTC - 1),
                    )
                p1_sb = wpool.tile([BPC, C], f32)
                nc.scalar.copy(p1_sb, p1_ps)

                p1t_sb = wpool.tile([128, NCH, BPC], f32)
                for ci in range(NCH):
                    t_ps = p0p.tile([128, BPC], f32, tag="p0")
                    nc.tensor.transpose(
                        t_ps, p1_sb[:, ci * 128:(ci + 1) * 128], ident[:BPC, :BPC]
                    )
                    nc.scalar.copy(p1t_sb[:, ci, :], t_ps)

                p2_ps = p0p.tile([BPC, C], f32, tag="p0")
                for ci in range(NCH):
                    nc.tensor.matmul(
                        p2_ps, p1t_sb[:, ci, :], wo_sb[:, ci, :],
                        start=(ci == 0), stop=(ci == NCH - 1),
                    )
                p_sb = wpool.tile([BPC, C], f32)
                nc.vector.tensor_add(p_sb, p2_ps, bo_sb)

                # p as columns: pcol_sb[:, ci, b] = p[b, ci*128 + :]
                for ci in range(NCH):
                    t_ps = p0p.tile([128, BPC], f32, tag="p0")
                    nc.tensor.transpose(
                        t_ps, p_sb[:, ci * 128:(ci + 1) * 128], ident[:BPC, :BPC]
                    )
                    nc.scalar.copy(pcol_sb[:, ci, :], t_ps)

                # -c1b row: -(sum_c p[b, c]) / C, shape (1, BPC)
                csum = wpool.tile([BPC, 1], f32)
                nc.vector.reduce_sum(csum, p_sb, axis=mybir.AxisListType.X)
                c1t_ps = p0p.tile([1, BPC], f32, tag="p0")
                nc.tensor.transpose(c1t_ps, csum, ident[:BPC, :BPC])
                nc.scalar.activation(nbias_row, c1t_ps, AF.Copy, scale=-1.0 / C)
                # broadcast -c1b[b] to partitions {0,32,64,96} for the packed
                # stats-row ops
                for b in range(BPC):
                    cb_ps = p0p.tile([97, 1], f32, tag="p0")
                    nc.tensor.matmul(
                        cb_ps, ones_m, nbias_row[:, b:b + 1], start=True, stop=True
                    )
                    nc.scalar.copy(nbias97[:, b:b + 1], cb_ps)

            if not trivial_affine:
                g_sb = consts.tile([128, NCH], f32)
                nc.sync.dma_start(g_sb, gcols)
                b_sb = consts.tile([128, NCH], f32)
                nc.sync.dma_start(b_sb, bcols)

            # ---------------- main loop ----------------
            with (
                tc.tile_pool(name="xp", bufs=2) as xp,
                tc.tile_pool(name="x16p", bufs=2) as x16p,
                tc.tile_pool(name="sqp", bufs=5) as sqp,
                tc.tile_pool(name="rowp", bufs=2) as rowp,
                tc.tile_pool(name="mup", bufs=2, space="PSUM") as mup,
                tc.tile_pool(name="e2p", bufs=2, space="PSUM") as e2p,
                tc.tile_pool(name="rstp", bufs=2, space="PSUM") as rstp,
                tc.tile_pool(name="hmp", bufs=2, space="PSUM") as hmp,
            ):
             for b in range(BPC):
                for m in range(NMACRO):
                    s0 = m * MACRO
                    xt = xp.tile([128, NCH, MACRO], f32)
                    nc.sync.dma_start(xt, xv[b, :, :, s0:s0 + MACRO])
                    x16 = x16p.tile([128, NCH, MACRO], bf16)
                    for ci in range(NCH):
                        nc.vector.tensor_copy(x16[:, ci, :], xt[:, ci, :])

                    # squares (x + p)^2 with the p-add folded into the ACT bias
                    sq_tiles = []
                    for ci in range(NCH):
                        sq = sqp.tile([128, MACRO], bf16, name=f"sq{ci}", tag="sq")
                        nc.scalar.activation(
                            sq, xt[:, ci, :], AF.Square,
                            bias=pcol_sb[:, ci, b:b + 1], scale=1.0,
                        )
                        sq_tiles.append(sq)

                    # channel sums for the whole macro tile: row j lives at
                    # partition 32*j of a single PSUM bank
                    mu_all = mup.tile([97, SUB], f32)
                    e2_all = e2p.tile([97, SUB], f32)
                    for j in range(NSUB):
                        sl = slice(SUB * j, SUB * (j + 1))
                        mrow = mu_all[32 * j:32 * j + 1, :]
                        for ci in range(NCH):
                            nc.tensor.matmul(
                                mrow, ones_c, x16[:, ci, sl],
                                start=(ci == 0), stop=(ci == NCH - 1),
                                tile_position=(0, 32 * j),
                            )
                    for j in range(NSUB):
                        sl = slice(SUB * j, SUB * (j + 1))
                        erow = e2_all[32 * j:32 * j + 1, :]
                        for ci in range(NCH):
                            nc.tensor.matmul(
                                erow, ones_c, sq_tiles[ci][:, sl],
                                start=(ci == 0), stop=(ci == NCH - 1),
                                tile_position=(0, 32 * j),
                            )

                    # stats finalize: one op per stage covering all 4 rows via
                    # partition-stride-32 APs
                    # stats ops run over all 97 partitions (only rows
                    # 0/32/64/96 are real; the rest compute garbage in
                    # parallel lanes at no extra cost)
                    negmu = rowp.tile([97, SUB], bf16, tag="negmu")
                    nc.scalar.activation(
                        negmu, mu_all, AF.Identity,
                        scale=-1.0, bias=nbias97[:, b:b + 1],
                    )
                    w = rowp.tile([97, SUB], f32, tag="w")
                    nc.scalar.activation(w, negmu, AF.Square)
                    nc.vector.tensor_tensor(w, e2_all, w, op=OP.subtract)
                    nc.scalar.activation(w, w, AF.Sqrt, bias=eps97)
                    rstd = rowp.tile([97, SUB], f32, tag="rstd")
                    rscr = rowp.tile([97, SUB], f32, tag="rscr")
                    nc.vector.reciprocal_approx_accurate(rstd, w, scratch=rscr)

                    # value phase: hm = x - mu in PSUM, then one fused
                    # (hm + p) * rstd per chunk; y overwrites xt in place
                    for j in range(NSUB):
                        sl = slice(SUB * j, SUB * (j + 1))
                        pr = 32 * j
                        rst_ps = rstp.tile([128, SUB], f32)
                        nc.tensor.matmul(
                            rst_ps, ones97[pr:pr + 1, :], rstd[pr:pr + 1, :],
                            start=True, stop=True, tile_position=(pr, 0),
                        )
                        rst_sb = rowp.tile([128, SUB], f32, tag="rst_sb")
                        nc.scalar.copy(rst_sb, rst_ps)
                        for ci in range(NCH):
                            hm_ps = hmp.tile([128, SUB], f32)
                            nc.tensor.matmul(
                                hm_ps, ident16, x16[:, ci, sl],
                                start=True, stop=False,
                            )
                            nc.tensor.matmul(
                                hm_ps, ones97_16[pr:pr + 1, :], negmu[pr:pr + 1, :],
                                start=False, stop=True, tile_position=(pr, 0),
                            )
                            out_sl = xt[:, ci, sl]
                            nc.vector.scalar_tensor_tensor(
                                out_sl, hm_ps, pcol_sb[:, ci, b:b + 1], rst_sb,
                                op0=OP.add, op1=OP.mult,
                            )
                            if not trivial_affine:
                                nc.vector.tensor_scalar(
                                    out_sl, out_sl,
                                    g_sb[:, ci:ci + 1], b_sb[:, ci:ci + 1],
                                    op0=OP.mult, op1=OP.add,
                                )

                    nc.sync.dma_start(yv[b, :, :, s0:s0 + MACRO], xt)

    nc.compile()
    return nc


def _get_module(trivial_affine: bool):
    key = trivial_affine
    if key not in _CACHE:
        _CACHE[key] = _build(trivial_affine)
    return _CACHE[key]


def kernel(**inputs) -> np.ndarray:
    global LAST_RESULTS
    from concourse.bass_utils import run_bass_kernel_spmd

    x = np.ascontiguousarray(np.asarray(inputs["x"], dtype=np.float32))
    te = np.asarray(inputs["text_emb"], dtype=np.float32)
    Wv = np.asarray(inputs["Wv"], dtype=np.float32)
    Wo = np.asarray(inputs["Wo"], dtype=np.float32)
    bo = np.asarray(inputs["bo"], dtype=np.float32)
    gamma = np.asarray(inputs["gamma"], dtype=np.float32)
    beta = np.asarray(inputs["beta"], dtype=np.float32)
    assert x.shape == (B, C, H, W), x.shape

    trivial = bool(np.all(gamma == 1.0) and np.all(beta == 0.0))
    nc = _get_module(trivial)

    xr = x.reshape(B, C, S)
    teT = np.ascontiguousarray(te.T)  # (T, B)
    wvT = np.ascontiguousarray(Wv.T)  # (T, C)
    woT = np.ascontiguousarray(Wo.T)  # (C, C)
    bo2 = np.ascontiguousarray(bo.reshape(1, C))

    in_maps = []
    for c in range(NCORES):
        m = {
            "x": np.ascontiguousarray(xr[BPC * c:BPC * (c + 1)]),
            "teT": np.ascontiguousarray(teT[:, BPC * c:BPC * (c + 1)]),
            "wvT": wvT,
            "woT": woT,
            "bo": bo2,
        }
        if not trivial:
            m["gcols"] = np.ascontiguousarray(gamma.reshape(NCH, 128).T)
            m["bcols"] = np.ascontiguousarray(beta.reshape(NCH, 128).T)
        in_maps.append(m)

    kwargs = {}
    if TRACE:
        import os

        os.makedirs("/tmp/bassprof", exist_ok=True)
        kwargs["tmpdir"] = "/tmp/bassprof"
    res = run_bass_kernel_spmd(
        nc, in_maps, core_ids=list(range(NCORES)), trace=TRACE, **kwargs
    )
    LAST_RESULTS = res
    out = np.concatenate([res.results[c]["y"] for c in range(NCORES)], axis=0)
    return np.ascontiguousarray(out.reshape(B, C, H, W).astype(np.float32))
